# revision 13
# baseline (speedup 1.0000x reference)
"""HSMNet cost-volume + disparity softmax-regression on 8 Trainium2 NeuronCores.

Reference computation (per batch b):
  cost[c,d,h,w] = |ref[c,h,w] - tgt[c,h,w-d]| for w>=d else 0
  cost_agg[d,h,w] = sum_c cost
  pred[h,w] = sum_d d * softmax_d(cost_agg)

Sharding: 8 cores = 4 batches x 2 h-halves (40 rows of 80 each). Each core
processes its [32, 40, 160] slice fully fused on-chip.

Host prep (layout only, no arithmetic): inputs are cast to fp16 and
replicated into 4 partition groups (partition = c + 32*j) with the shift j
baked into tgt via a 24-col front zero pad. On-chip, per eighth of the
pixel range (800 pixels):
  - one DVE tensor_tensor subtract with a 3D access pattern (disparity
    block dim stride +4 on tgt, stride 0 broadcast on ref) produces diffs
    for all 24 disparities: diff[c+32j, k, p] = ref[c,p] - tgt[c, p-4b-j],
    b = 5-k.
  - abs in place, split across DVE (uint16 bitand), ACT (Abs), GPSIMD
    (uint16 bitand) per env-tunable column split.
  - TensorE reduces channels with 0/1 weights into PSUM [24, 2x512], plus
    one extra accumulation matmul that adds -10000 where w < d (validity
    mask folded into the PE pass: [w<d] = sum_k [k<d]*[w==k]).
  - ACT Exp evacuates PSUM -> E[96, 1600] bf16 (rows 24q+d).
  - TensorE contracts E with [ones; d] weights -> den/num [8, 1600].
  - host divides num/den (invalid entries' terms vanish: exp(-1e4) = 0).
"""
import os
import sys
import threading

for _p in ("/opt/trn_rl_repo",):
    if os.path.isdir(_p) and _p not in sys.path:
        sys.path.insert(0, _p)

import numpy as np
import ml_dtypes

import concourse.bacc as bacc
import concourse.mybir as mybir
from concourse.tile import TileContext
from concourse.bass_utils import run_bass_kernel_spmd

dt = mybir.dt

# problem shape (hardcoded per spec)
B, C, H, W = 4, 32, 80, 160
D = 24
HP = H // 2            # rows per core
PIX = HP * W           # 6400 pixels per core
NB = D // 4            # 6 disparity blocks of 4
PAD = 24               # zero pad columns in front of tgtr
NE = 8                 # processing units (eighths of the pixel range)
EW = PIX // NE         # 800 pixels per eighth
QW = PIX // 4          # 1600 pixels per quarter (E column range)
N_CORES = 8

# abs column split within each [128, 4800] diff tile: [0:A) DVE bitand,
# [A:B) ACT Abs, [B:4800) GPSIMD bitand. Multiples of 16.
ABS_DVE = int(os.environ.get("HSM_ABS_DVE", "2400"))
ABS_ACT = int(os.environ.get("HSM_ABS_ACT", "2400"))
DIFF_BUFS = int(os.environ.get("HSM_DIFF_BUFS", "3"))
OUT_VIA_ACT = int(os.environ.get("HSM_OUT_ACT", "1"))


def _build_program():
    nc = bacc.Bacc("TRN2", target_bir_lowering=False)
    refr_h = nc.dram_tensor("refr", [128, PIX], dt.float16, kind="ExternalInput")
    tgtr_h = nc.dram_tensor("tgtr", [128, PAD + PIX], dt.float16,
                            kind="ExternalInput")
    lred_h = nc.dram_tensor("lred", [128, NB * D], dt.float16,
                            kind="ExternalInput")
    lmask_h = nc.dram_tensor("lmask", [D, D], dt.float16, kind="ExternalInput")
    maskc_h = nc.dram_tensor("maskc", [D, EW], dt.float16, kind="ExternalInput")
    lnd_h = nc.dram_tensor("lnd", [128, 8], dt.bfloat16, kind="ExternalInput")
    out_h = nc.dram_tensor("out", [8, 4 * 400], dt.float32, kind="ExternalOutput")

    with TileContext(nc) as tc:
        with tc.tile_pool(name="const", bufs=1) as cpool, \
             tc.tile_pool(name="inp", bufs=NE) as ipool, \
             tc.tile_pool(name="diffp", bufs=DIFF_BUFS) as dpool, \
             tc.tile_pool(name="ep", bufs=1) as epool:
            lred_sb = cpool.tile([128, NB * D], dt.float16)
            lmask_sb = cpool.tile([D, D], dt.float16)
            maskc_sb = cpool.tile([D, EW], dt.float16)
            lnd_sb = cpool.tile([128, 8], dt.bfloat16)
            nc.sync.dma_start(lred_sb[:], lred_h[:])
            nc.sync.dma_start(lmask_sb[:], lmask_h[:])
            nc.sync.dma_start(maskc_sb[:], maskc_h[:])
            nc.sync.dma_start(lnd_sb[:], lnd_h[:])

            E = epool.tile([128, QW], dt.bfloat16)
            # rows 32q+24..32q+31 are never written by the exp evac; zero
            # them once so the num/den matmul sees 0 (their lnd weights are
            # 0, but garbage could be inf/nan)
            nc.gpsimd.memset(E[:], 0.0)

            # per-eighth input tiles so the first subtract only waits for
            # its own chunk's DMA, not the whole input load. tgt chunks
            # overlap by PAD cols: chunk e covers tgtr_h cols
            # [EW*e, EW*e + PAD + EW) = original tgt cols [EW*e-24, EW*e+800)
            refs = {}
            tgts = {}

            def emit_load(e):
                c0 = EW * e
                rt = ipool.tile([128, EW], dt.float16, tag="refc",
                                name=f"refc_{e}")
                nc.sync.dma_start(rt[:], refr_h[:, c0:c0 + EW])
                tt_ = ipool.tile([128, PAD + EW], dt.float16, tag="tgtc",
                                 name=f"tgtc_{e}")
                nc.sync.dma_start(tt_[:], tgtr_h[:, c0:c0 + PAD + EW])
                refs[e], tgts[e] = rt, tt_

            diffs = {}

            def emit_tt(e):
                diff = dpool.tile([128, NB * EW], dt.float16, tag="diff",
                                  name=f"diff_{e}")
                out = diff[:].rearrange("p (k x) -> p k x", x=EW)
                in0 = refs[e][:].unsqueeze(1).broadcast_to([128, NB, EW])
                # tgt windows: block k reads chunk cols [4+4k, 4+4k+EW)
                # -> diff slot k holds disparity block b = 5-k
                in1 = tgts[e][:, 4:4 + EW].unsqueeze(1).broadcast_to(
                    [128, NB, EW]).copy()
                in1.ap = in1.ap[:1] + (((4, NB)),) + in1.ap[2:]
                nc.vector.tensor_tensor(out, in0, in1,
                                        mybir.AluOpType.subtract)
                diffs[e] = diff

            def emit_abs(e):
                diff = diffs[e]
                if ABS_DVE > 0:
                    du = diff[:, 0:ABS_DVE].bitcast(dt.uint16)
                    nc.vector.tensor_scalar(du, du, 0x7FFF, None,
                                            mybir.AluOpType.bitwise_and)
                if ABS_ACT > 0:
                    s0, s1 = ABS_DVE, ABS_DVE + ABS_ACT
                    nc.scalar.activation(diff[:, s0:s1], diff[:, s0:s1],
                                         mybir.ActivationFunctionType.Abs)
                if ABS_DVE + ABS_ACT < NB * EW:
                    s0 = ABS_DVE + ABS_ACT
                    g = diff[:, s0:NB * EW]
                    nc.gpsimd.scalar_tensor_tensor(
                        g, g, -1.0, g,
                        op0=mybir.AluOpType.mult, op1=mybir.AluOpType.max)

            costs = {}

            def emit_pe(e, qpool):
                diff = diffs[e]
                cost = qpool.tile([D, 1024], dt.float32, tag="cost",
                                  name=f"cost_{e}")
                for k in range(NB):
                    b = NB - 1 - k
                    for cc in range(2):
                        nc.tensor.matmul(
                            cost[:, 512 * cc:512 * cc + 400],
                            lred_sb[:, D * b:D * (b + 1)],
                            diff[:, EW * k + 400 * cc:EW * k + 400 * cc + 400],
                            start=(k == 0), stop=False)
                for cc in range(2):
                    nc.tensor.matmul(
                        cost[:, 512 * cc:512 * cc + 400],
                        lmask_sb[:],
                        maskc_sb[:, 400 * cc:400 * cc + 400],
                        start=False, stop=(cc == 1))
                costs[e] = cost

            def emit_exp(e):
                q, hh = e // 2, e % 2
                cost = costs[e]
                src = cost[:].rearrange("p (k x) -> p k x", x=512)[:, :, 0:400]
                dst = E[32 * q:32 * q + D,
                        EW * hh:EW * (hh + 1)].rearrange(
                            "p (k x) -> p k x", x=400)
                nc.scalar.activation(dst, src, mybir.ActivationFunctionType.Exp)
                del costs[e], diffs[e]

            with tc.tile_pool(name="cost", bufs=4, space="PSUM") as qpool:
                for e in range(NE):
                    emit_load(e)
                    emit_tt(e)
                    emit_abs(e)
                    if e >= 1:
                        emit_pe(e - 1, qpool)
                        emit_exp(e - 1)
                emit_pe(NE - 1, qpool)
                emit_exp(NE - 1)

            with tc.tile_pool(name="nd", bufs=1, space="PSUM") as npool:
                nd = npool.tile([8, 2048], dt.float32)
                for cc in range(4):
                    nc.tensor.matmul(nd[:, 512 * cc:512 * cc + 400],
                                     lnd_sb[:], E[:, 400 * cc:400 * (cc + 1)],
                                     start=True, stop=True)
                ndsrc = nd[:].rearrange("p (k x) -> p k x", x=512)[:, :, 0:400]
                if OUT_VIA_ACT:
                    out_sb = epool.tile([8, 4 * 400], dt.float32)
                    nc.scalar.activation(
                        out_sb[:].rearrange("p (k x) -> p k x", x=400), ndsrc,
                        mybir.ActivationFunctionType.Copy)
                    nc.sync.dma_start(out_h[:], out_sb[:])
                else:
                    nc.sync.dma_start(
                        out_h[:].rearrange("p (k x) -> p k x", x=400), ndsrc)

    nc.compile()
    return nc


def _host_constants():
    # lred: block b sums channels of partition group j into cost row 4b+j
    lred = np.zeros((128, NB * D), np.float16)
    for b in range(NB):
        for j in range(4):
            for c in range(C):
                lred[c + 32 * j, D * b + 4 * b + j] = 1.0

    # bias[d, p] = sum_k lmask[k, d] * maskc[k, p] = -1e4 * [(p mod W) < d]
    lmask = np.zeros((D, D), np.float16)
    for k in range(D):
        for d in range(D):
            if k < d:
                lmask[k, d] = 1.0
    maskc = np.zeros((D, EW), np.float16)
    for k in range(D):
        maskc[k, np.arange(EW)[np.arange(EW) % W == k]] = -10000.0

    lnd = np.zeros((128, 8), np.float32)
    for q in range(4):
        for d in range(D):
            lnd[d + 32 * q, q] = 1.0      # den
            lnd[d + 32 * q, 4 + q] = d    # num
    lnd = lnd.astype(ml_dtypes.bfloat16)
    return lred, lmask, maskc, lnd


_lock = threading.Lock()
_cache = {}


def _get_program():
    with _lock:
        if "nc" not in _cache:
            _cache["nc"] = _build_program()
            _cache["consts"] = _host_constants()
        return _cache["nc"], _cache["consts"]


def _prep_core(ref_s, tgt_s):
    """ref_s, tgt_s: [32, 6400] fp16 -> replicated tiles."""
    refr = np.ascontiguousarray(
        np.broadcast_to(ref_s[None], (4, C, PIX)).reshape(128, PIX))
    tgtr = np.zeros((128, PAD + PIX), np.float16)
    for j in range(4):
        tgtr[32 * j:32 * j + 32, PAD + j:] = tgt_s[:, :PIX - j]
    return refr, tgtr


def _run(refimg_fea, targetimg_fea, trace=False):
    nc, (lred, lmask, maskc, lnd) = _get_program()
    ref = np.asarray(refimg_fea, dtype=np.float32).astype(np.float16)
    tgt = np.asarray(targetimg_fea, dtype=np.float32).astype(np.float16)
    in_maps = []
    for core in range(N_CORES):
        b, hh = core // 2, core % 2
        ref_s = ref[b, :, HP * hh:HP * (hh + 1), :].reshape(C, PIX)
        tgt_s = tgt[b, :, HP * hh:HP * (hh + 1), :].reshape(C, PIX)
        refr, tgtr = _prep_core(ref_s, tgt_s)
        in_maps.append({
            "refr": refr, "tgtr": tgtr,
            "lred": lred, "lmask": lmask, "maskc": maskc, "lnd": lnd,
        })
    res = run_bass_kernel_spmd(nc, in_maps, core_ids=list(range(N_CORES)),
                               trace=trace)
    out = np.empty((B, H, W), np.float32)
    for core in range(N_CORES):
        b, hh = core // 2, core % 2
        nd = res.results[core]["out"]          # [8, 1600]: den rows 0-3, num 4-7
        pred = nd[4:8] / nd[0:4]               # [4, 1600]
        out[b, HP * hh:HP * (hh + 1), :] = pred.reshape(HP, W)
    return out, res


def kernel(refimg_fea, targetimg_fea, maxdisp):
    assert int(maxdisp) == D, f"kernel hardcodes maxdisp={D}, got {maxdisp}"
    out, _ = _run(refimg_fea, targetimg_fea)
    return out


# revision 18
# speedup vs baseline: 1.0484x; 1.0484x over previous
"""HSMNet cost-volume + disparity softmax-regression on 8 Trainium2 NeuronCores.

Reference computation (per batch b):
  cost[c,d,h,w] = |ref[c,h,w] - tgt[c,h,w-d]| for w>=d else 0
  cost_agg[d,h,w] = sum_c cost
  pred[h,w] = sum_d d * softmax_d(cost_agg)

Sharding: 8 cores = 4 batches x 2 h-halves (40 rows of 80 each). Each core
processes its [32, 40, 160] slice fully fused on-chip.

Host prep (layout only, no arithmetic): inputs are cast to fp16 and
replicated into 4 partition groups (partition = c + 32*j) with the shift j
baked into tgt via a 24-col front zero pad. On-chip, per eighth of the
pixel range (800 pixels):
  - one DVE tensor_tensor subtract with a 3D access pattern (disparity
    block dim stride +4 on tgt, stride 0 broadcast on ref) produces diffs
    for all 24 disparities: diff[c+32j, k, p] = ref[c,p] - tgt[c, p-4b-j],
    b = 5-k.
  - abs in place, split across DVE (uint16 bitand), ACT (Abs), GPSIMD
    (uint16 bitand) per env-tunable column split.
  - TensorE reduces channels with 0/1 weights into PSUM [24, 2x512], plus
    one extra accumulation matmul that adds -10000 where w < d (validity
    mask folded into the PE pass: [w<d] = sum_k [k<d]*[w==k]).
  - ACT Exp evacuates PSUM -> E[96, 1600] bf16 (rows 24q+d).
  - TensorE contracts E with [ones; d] weights -> den/num [8, 1600].
  - host divides num/den (invalid entries' terms vanish: exp(-1e4) = 0).
"""
import os
import sys
import threading

for _p in ("/opt/trn_rl_repo",):
    if os.path.isdir(_p) and _p not in sys.path:
        sys.path.insert(0, _p)

import numpy as np
import ml_dtypes

import concourse.bacc as bacc
import concourse.mybir as mybir
from concourse.tile import TileContext
from concourse.bass_utils import run_bass_kernel_spmd

dt = mybir.dt

# problem shape (hardcoded per spec)
B, C, H, W = 4, 32, 80, 160
D = 24
HP = H // 2            # rows per core
PIX = HP * W           # 6400 pixels per core
NB = D // 4            # 6 disparity blocks of 4
PAD = 24               # zero pad columns in front of tgtr
NE = 8                 # processing units (eighths of the pixel range)
EW = PIX // NE         # 800 pixels per eighth
QW = PIX // 4          # 1600 pixels per quarter (E column range)
N_CORES = 8

# abs column split within each [128, 4800] diff tile: [0:A) DVE bitand,
# [A:B) ACT Abs, [B:4800) GPSIMD bitand. Multiples of 16.
ABS_DVE = int(os.environ.get("HSM_ABS_DVE", "2560"))
ABS_ACT = int(os.environ.get("HSM_ABS_ACT", "2240"))
DIFF_BUFS = int(os.environ.get("HSM_DIFF_BUFS", "3"))
COST_BUFS = int(os.environ.get("HSM_COST_BUFS", "2"))
OUT_VIA_ACT = int(os.environ.get("HSM_OUT_ACT", "1"))


def _build_program():
    nc = bacc.Bacc("TRN2", target_bir_lowering=False)
    refr_h = nc.dram_tensor("refr", [128, PIX], dt.float16, kind="ExternalInput")
    tgtr_h = nc.dram_tensor("tgtr", [128, PAD + PIX], dt.float16,
                            kind="ExternalInput")
    lred_h = nc.dram_tensor("lred", [128, NB * D], dt.float16,
                            kind="ExternalInput")
    lmask_h = nc.dram_tensor("lmask", [D, D], dt.float16, kind="ExternalInput")
    maskc_h = nc.dram_tensor("maskc", [D, EW], dt.float16, kind="ExternalInput")
    lnd_h = nc.dram_tensor("lnd", [128, 8], dt.bfloat16, kind="ExternalInput")
    out_h = nc.dram_tensor("out", [8, 4 * 400], dt.float32, kind="ExternalOutput")

    with TileContext(nc) as tc:
        with tc.tile_pool(name="const", bufs=1) as cpool, \
             tc.tile_pool(name="inp", bufs=4) as ipool, \
             tc.tile_pool(name="diffp", bufs=DIFF_BUFS) as dpool, \
             tc.tile_pool(name="ep", bufs=1) as epool:
            lred_sb = cpool.tile([128, NB * D], dt.float16)
            lmask_sb = cpool.tile([D, D], dt.float16)
            maskc_sb = cpool.tile([D, EW], dt.float16)
            lnd_sb = cpool.tile([128, 8], dt.bfloat16)

            E = epool.tile([128, QW], dt.bfloat16)

            # per-quarter input tiles (first quarter's loads issued first,
            # on the sync queue; the rest on the idle gpsimd queue so the
            # ~0.6us/DMA descriptor-gen doesn't serialize ahead of quarter
            # 0). tgt quarter q covers tgtr_h cols [QW*q, QW*q + PAD + QW).
            refs = {}
            tgts = {}

            def emit_load(q, eng):
                c0 = QW * q
                rt = ipool.tile([128, QW], dt.float16, tag="refc",
                                name=f"refc_{q}")
                eng.dma_start(rt[:], refr_h[:, c0:c0 + QW])
                tt_ = ipool.tile([128, PAD + QW], dt.float16, tag="tgtc",
                                 name=f"tgtc_{q}")
                eng.dma_start(tt_[:], tgtr_h[:, c0:c0 + PAD + QW])
                refs[q], tgts[q] = rt, tt_

            emit_load(0, nc.sync)
            nc.sync.dma_start(lred_sb[:], lred_h[:])
            nc.sync.dma_start(lmask_sb[:], lmask_h[:])
            nc.sync.dma_start(maskc_sb[:], maskc_h[:])
            nc.sync.dma_start(lnd_sb[:], lnd_h[:])
            # rows 32q+24..32q+31 are never written by the exp evac; zero
            # them once so the num/den matmul sees 0 (their lnd weights are
            # 0, but garbage could be inf/nan)
            nc.gpsimd.memset(E[:], 0.0)
            for q in range(1, 4):
                emit_load(q, nc.gpsimd)

            diffs = {}

            def emit_tt(e):
                q, hh = e // 2, e % 2
                diff = dpool.tile([128, NB * EW], dt.float16, tag="diff",
                                  name=f"diff_{e}")
                out = diff[:].rearrange("p (k x) -> p k x", x=EW)
                in0 = refs[q][:, EW * hh:EW * hh + EW].unsqueeze(
                    1).broadcast_to([128, NB, EW])
                # tgt windows: block k reads quarter cols
                # [EW*hh+4+4k, EW*hh+4+4k+EW) -> diff slot k holds block 5-k
                in1 = tgts[q][:, EW * hh + 4:EW * hh + 4 + EW].unsqueeze(
                    1).broadcast_to([128, NB, EW]).copy()
                in1.ap = in1.ap[:1] + (((4, NB)),) + in1.ap[2:]
                nc.vector.tensor_tensor(out, in0, in1,
                                        mybir.AluOpType.subtract)
                diffs[e] = diff

            def emit_abs(e):
                diff = diffs[e]
                if ABS_DVE > 0:
                    du = diff[:, 0:ABS_DVE].bitcast(dt.uint16)
                    nc.vector.tensor_scalar(du, du, 0x7FFF, None,
                                            mybir.AluOpType.bitwise_and)
                if ABS_ACT > 0:
                    s0, s1 = ABS_DVE, ABS_DVE + ABS_ACT
                    nc.scalar.activation(diff[:, s0:s1], diff[:, s0:s1],
                                         mybir.ActivationFunctionType.Abs)
                if ABS_DVE + ABS_ACT < NB * EW:
                    s0 = ABS_DVE + ABS_ACT
                    g = diff[:, s0:NB * EW]
                    nc.gpsimd.scalar_tensor_tensor(
                        g, g, -1.0, g,
                        op0=mybir.AluOpType.mult, op1=mybir.AluOpType.max)

            costs = {}

            def emit_pe(e, qpool):
                diff = diffs[e]
                cost = qpool.tile([D, 1024], dt.float32, tag="cost",
                                  name=f"cost_{e}")
                for k in range(NB):
                    b = NB - 1 - k
                    for cc in range(2):
                        nc.tensor.matmul(
                            cost[:, 512 * cc:512 * cc + 400],
                            lred_sb[:, D * b:D * (b + 1)],
                            diff[:, EW * k + 400 * cc:EW * k + 400 * cc + 400],
                            start=(k == 0), stop=False)
                for cc in range(2):
                    nc.tensor.matmul(
                        cost[:, 512 * cc:512 * cc + 400],
                        lmask_sb[:],
                        maskc_sb[:, 400 * cc:400 * cc + 400],
                        start=False, stop=(cc == 1))
                costs[e] = cost

            def emit_exp(e):
                q, hh = e // 2, e % 2
                cost = costs[e]
                src = cost[:].rearrange("p (k x) -> p k x", x=512)[:, :, 0:400]
                dst = E[32 * q:32 * q + D,
                        EW * hh:EW * (hh + 1)].rearrange(
                            "p (k x) -> p k x", x=400)
                nc.scalar.activation(dst, src, mybir.ActivationFunctionType.Exp)
                del costs[e], diffs[e]

            with tc.tile_pool(name="cost", bufs=COST_BUFS, space="PSUM") as qpool, \
                 tc.tile_pool(name="nd", bufs=1, space="PSUM") as npool:
                nd = npool.tile([8, 2048], dt.float32)

                def emit_nd(half):
                    # partial num/den contraction over quarters 2h, 2h+1
                    # (PE operands cannot start at partition 96, so halves)
                    for cc in range(4):
                        nc.tensor.matmul(
                            nd[:, 512 * cc:512 * cc + 400],
                            lnd_sb[64 * half:64 * half + 64, :],
                            E[64 * half:64 * half + 64,
                              400 * cc:400 * (cc + 1)],
                            start=(half == 0), stop=(half == 1))

                for e in range(NE):
                    emit_tt(e)
                    emit_abs(e)
                    if e >= 1:
                        emit_pe(e - 1, qpool)
                        emit_exp(e - 1)
                        if e == 5:
                            emit_nd(0)
                emit_pe(NE - 1, qpool)
                emit_exp(NE - 1)
                emit_nd(1)

                ndsrc = nd[:].rearrange("p (k x) -> p k x", x=512)[:, :, 0:400]
                out_sb = epool.tile([8, 4 * 400], dt.float32)
                nc.scalar.activation(
                    out_sb[:].rearrange("p (k x) -> p k x", x=400), ndsrc,
                    mybir.ActivationFunctionType.Copy)
                nc.sync.dma_start(out_h[:], out_sb[:])

    nc.compile()
    return nc


def _host_constants():
    # lred: block b sums channels of partition group j into cost row 4b+j
    lred = np.zeros((128, NB * D), np.float16)
    for b in range(NB):
        for j in range(4):
            for c in range(C):
                lred[c + 32 * j, D * b + 4 * b + j] = 1.0

    # bias[d, p] = sum_k lmask[k, d] * maskc[k, p] = -1e4 * [(p mod W) < d]
    lmask = np.zeros((D, D), np.float16)
    for k in range(D):
        for d in range(D):
            if k < d:
                lmask[k, d] = 1.0
    maskc = np.zeros((D, EW), np.float16)
    for k in range(D):
        maskc[k, np.arange(EW)[np.arange(EW) % W == k]] = -10000.0

    lnd = np.zeros((128, 8), np.float32)
    for q in range(4):
        for d in range(D):
            lnd[d + 32 * q, q] = 1.0      # den
            lnd[d + 32 * q, 4 + q] = d    # num
    lnd = lnd.astype(ml_dtypes.bfloat16)
    return lred, lmask, maskc, lnd


_lock = threading.Lock()
_cache = {}


def _get_program():
    with _lock:
        if "nc" not in _cache:
            _cache["nc"] = _build_program()
            _cache["consts"] = _host_constants()
        return _cache["nc"], _cache["consts"]


def _prep_core(ref_s, tgt_s):
    """ref_s, tgt_s: [32, 6400] fp16 -> replicated tiles."""
    refr = np.ascontiguousarray(
        np.broadcast_to(ref_s[None], (4, C, PIX)).reshape(128, PIX))
    tgtr = np.zeros((128, PAD + PIX), np.float16)
    for j in range(4):
        tgtr[32 * j:32 * j + 32, PAD + j:] = tgt_s[:, :PIX - j]
    return refr, tgtr


def _run(refimg_fea, targetimg_fea, trace=False):
    nc, (lred, lmask, maskc, lnd) = _get_program()
    ref = np.asarray(refimg_fea, dtype=np.float32).astype(np.float16)
    tgt = np.asarray(targetimg_fea, dtype=np.float32).astype(np.float16)
    in_maps = []
    for core in range(N_CORES):
        b, hh = core // 2, core % 2
        ref_s = ref[b, :, HP * hh:HP * (hh + 1), :].reshape(C, PIX)
        tgt_s = tgt[b, :, HP * hh:HP * (hh + 1), :].reshape(C, PIX)
        refr, tgtr = _prep_core(ref_s, tgt_s)
        in_maps.append({
            "refr": refr, "tgtr": tgtr,
            "lred": lred, "lmask": lmask, "maskc": maskc, "lnd": lnd,
        })
    res = run_bass_kernel_spmd(nc, in_maps, core_ids=list(range(N_CORES)),
                               trace=trace)
    out = np.empty((B, H, W), np.float32)
    for core in range(N_CORES):
        b, hh = core // 2, core % 2
        nd = res.results[core]["out"]          # [8, 1600]: den rows 0-3, num 4-7
        pred = nd[4:8] / nd[0:4]               # [4, 1600]
        out[b, HP * hh:HP * (hh + 1), :] = pred.reshape(HP, W)
    return out, res


def kernel(refimg_fea, targetimg_fea, maxdisp):
    assert int(maxdisp) == D, f"kernel hardcodes maxdisp={D}, got {maxdisp}"
    out, _ = _run(refimg_fea, targetimg_fea)
    return out
